# revision 6
# baseline (speedup 1.0000x reference)
"""CodeGEMMLinear (AQLM-style 2x8 VQ codebook linear) on 8 Trainium2 cores.

Strategy (column-parallel over out_features, x replicated):
  per core (o-shard = 512 columns):
    dequant:  for each (codebook c, in-vector-block p): replicate the 512
              uint8 codes (as exact bf16) across 128 partitions via a
              broadcast DMA, build the 256-way one-hot with two DVE
              `is_equal` ops against per-partition j columns, and contract
              one-hot x codebook on the PE (K=128 j-half, M=8 vec-lane,
              accumulated over c and j-half in PSUM).  Four p-blocks share
              one PSUM tile via 32-column-strip tile positions.  ACT
              evacuates PSUM -> bf16, a SBUF->SBUF DMA spreads the 8
              vec-lane partitions into the (v,pl)-ordered K-tile layout,
              and DVE applies the per-(group,o) scales.
    GEMM:     y^T[o,t] = W_ktile^T @ x^T on the PE in bf16 (K-tiles of 128,
              N=512 token chunks), f32 PSUM accumulation over 32 K-tiles.
  x^T is fed in a host-permuted row order so the dequant spread layout and
  the GEMM contraction layout agree; the permutation is pure layout.
"""
import copy
import numpy as np
import ml_dtypes

import concourse.bass as bass
import concourse.mybir as mybir
import concourse.tile as tile
from concourse.bass_utils import run_bass_kernel_spmd

# problem constants (hardcoded per harness contract)
T = 2048          # tokens
IN_F = 4096       # in features
OUT_F = 4096      # out features
C = 2             # codebooks
V = 8             # vec len
CBN = 256         # codebook entries
GS = 128          # scale group size
NCORE = 8
OS = OUT_F // NCORE   # 512 out features per core
P = IN_F // V         # 512 vector blocks
KT = IN_F // 128      # 32 K-tiles
PL = 16               # p-blocks per K-tile

BF16 = mybir.dt.bfloat16
F32 = mybir.dt.float32


def _split_waits(nc, max_waits=1):
    """This container's walrus rejects most instructions with >1 sync wait;
    hoist extra waits onto single-wait NOPs on the same engine (FIFO order
    preserves blocking semantics)."""
    for fn in nc.m.functions:
        for bb in fn.blocks:
            new_insts = []
            for inst in bb.instructions:
                si = inst.sync_info
                if si is not None and si.on_wait and len(si.on_wait) > max_waits:
                    waits = list(si.on_wait)
                    chunks = [waits[i:i + max_waits]
                              for i in range(0, len(waits), max_waits)]
                    for ci, chunk in enumerate(chunks[:-1]):
                        ni = mybir.InstNoOp(
                            name=f'{inst.name}-presplit{ci}',
                            sync_info=mybir.SyncInfo(on_wait=chunk, on_update=[]),
                            bass_nofuse=True,
                            engine=inst.engine,
                        )
                        new_insts.append(ni)
                        nc.register_instruction(ni, overwrite=True)
                    si.on_wait = chunks[-1]
                new_insts.append(inst)
            bb.instructions[:] = new_insts


def _build():
    nc = bass.Bass(target_bir_lowering=False)

    d_idx = nc.declare_dram_parameter("idxsrc", [C * P, OS], BF16, isOutput=False)
    d_xt = nc.declare_dram_parameter("xtp", [IN_F, T], F32, isOutput=False)
    d_cb = nc.declare_dram_parameter("cb", [C * CBN, V], F32, isOutput=False)
    d_jc = nc.declare_dram_parameter("jcol", [128, 2], F32, isOutput=False)
    d_sc = nc.declare_dram_parameter("scales", [KT, OS], F32, isOutput=False)
    d_y = nc.declare_dram_parameter("yT", [OS, T], F32, isOutput=True)

    with tile.TileContext(nc) as tc:
        with tc.tile_pool(name="const", bufs=1) as cpool, \
             tc.tile_pool(name="wall", bufs=1) as wpool, \
             tc.tile_pool(name="xt", bufs=1) as xpool, \
             tc.tile_pool(name="repl", bufs=4) as rpool, \
             tc.tile_pool(name="oh", bufs=3) as ohpool, \
             tc.tile_pool(name="wev", bufs=2) as wevpool, \
             tc.tile_pool(name="sc", bufs=2) as scpool, \
             tc.tile_pool(name="yev", bufs=2) as ypool, \
             tc.tile_pool(name="psw", bufs=4, space="PSUM") as pswpool, \
             tc.tile_pool(name="psy", bufs=4, space="PSUM") as psypool:

            # ---- constants ----
            t_cbf = cpool.tile([128, 2 * C * V], F32)   # 4 j-half slices side by side
            t_cb = cpool.tile([128, 2 * C * V], BF16)
            for c in range(C):
                for jh in range(2):
                    sl = slice((c * 2 + jh) * V, (c * 2 + jh + 1) * V)
                    nc.sync.dma_start(t_cbf[:, sl], d_cb[c * CBN + jh * 128: c * CBN + (jh + 1) * 128, :])
            nc.vector.tensor_copy(t_cb[:], t_cbf[:])
            t_jc = cpool.tile([128, 2], F32)
            nc.sync.dma_start(t_jc[:], d_jc[:])

            # persistent W (bf16, spread layout: partition nu = pl*8+v)
            w_all = wpool.tile([128, KT * OS], BF16)     # 32 KB/partition

            # ---- dequant ----
            KB = 4                       # K-tiles per wev/spread batch
            for kb in range(KT // KB):
                # wev free layout: (ktl, g, o); partitions 32j+v hold p-block 4g+j
                t_wev = wevpool.tile([128, KB * 4 * OS], BF16, tag="wev")
                for ktl in range(KB):
                    kt = kb * KB + ktl
                    # replicate code rows in two half-K-tile batches per codebook
                    t_repl = {}
                    for c in range(C):
                        for ph in range(2):
                            t_repl[c, ph] = rpool.tile([128, 8 * OS], BF16, tag="repl", name=f"trepl{c}{ph}")
                            r0 = c * P + kt * PL + 8 * ph
                            nc.sync.dma_start(
                                t_repl[c, ph][:],
                                d_idx[r0: r0 + 8, :].partition_broadcast(128))
                    for g in range(4):      # group of 4 p-blocks -> one PSUM tile
                        t_ps = pswpool.tile([128, OS], F32, tag="psw")
                        for j in range(4):  # column strip = p-block 4g+j
                            pl = 4 * g + j
                            for ci in range(4):  # (c, jh) accumulation
                                c, jh = divmod(ci, 2)
                                t_oh = ohpool.tile([128, OS], BF16, tag="oh")
                                nc.vector.tensor_scalar(
                                    t_oh[:],
                                    t_repl[c, pl // 8][:, (pl % 8) * OS:(pl % 8 + 1) * OS],
                                    t_jc[:, jh:jh + 1], None, mybir.AluOpType.is_equal)
                                nc.tensor.matmul(
                                    t_ps[32 * j:32 * j + V, :],
                                    t_cb[:, (c * 2 + jh) * V:(c * 2 + jh + 1) * V],
                                    t_oh[:],
                                    start=(ci == 0), stop=(ci == 3),
                                    tile_position=(0, 32 * j))
                        # evacuate all 4 strips (f32 PSUM -> bf16 SBUF) on ACT
                        nc.scalar.copy(
                            t_wev[:, (ktl * 4 + g) * OS:(ktl * 4 + g + 1) * OS],
                            t_ps[:])
                # spread (batched over KB k-tiles): strip 32j+v of group g
                #   -> w_all partition (4g+j)*8+v, free (kt, o)
                wev3 = t_wev[:].rearrange("p (k g o) -> p k g o", k=KB, g=4)
                wal3 = w_all[:].rearrange("p (k o) -> p k o", k=KT)
                for g in range(4):
                    for j in range(4):
                        pl = 4 * g + j
                        nc.sync.dma_start(
                            wal3[pl * V:(pl + 1) * V, kb * KB:(kb + 1) * KB, :],
                            wev3[32 * j:32 * j + V, :, g, :])
                # scales for this batch
                for ktl in range(KB):
                    kt = kb * KB + ktl
                    t_scf = scpool.tile([128, OS], F32, tag="scf")
                    nc.sync.dma_start(t_scf[:], d_sc[kt:kt + 1, :].partition_broadcast(128))
                    t_scb = scpool.tile([128, OS], BF16, tag="scb")
                    nc.vector.tensor_copy(t_scb[:], t_scf[:])
                    nc.vector.tensor_tensor(
                        w_all[:, kt * OS:(kt + 1) * OS],
                        w_all[:, kt * OS:(kt + 1) * OS], t_scb[:],
                        mybir.AluOpType.mult)

            # ---- GEMM ----
            TH = 1024
            for th in range(2):
                t_xt = xpool.tile([128, KT * TH], BF16, tag="xt")   # 64 KB/partition
                for kt in range(KT):
                    nc.gpsimd.dma_start(      # SWDGE cast f32 -> bf16
                        t_xt[:, kt * TH:(kt + 1) * TH],
                        d_xt[kt * 128:(kt + 1) * 128, th * TH:(th + 1) * TH])
                for tc_ in range(2):
                    for ob in range(4):
                        t_py = psypool.tile([128, 512], F32, tag="psy")
                        for kt in range(KT):
                            nc.tensor.matmul(
                                t_py[:],
                                w_all[:, kt * OS + ob * 128: kt * OS + (ob + 1) * 128],
                                t_xt[:, kt * TH + tc_ * 512: kt * TH + tc_ * 512 + 512],
                                start=(kt == 0), stop=(kt == KT - 1))
                        t_ye = ypool.tile([128, 512], F32, tag="ye")
                        nc.scalar.copy(t_ye[:], t_py[:])
                        nc.sync.dma_start(
                            d_y[ob * 128:(ob + 1) * 128,
                                th * TH + tc_ * 512: th * TH + tc_ * 512 + 512],
                            t_ye[:])

    _split_waits(nc)
    return nc


_NC_CACHE = None


def kernel(x, codes, codebooks, scales, group_size):
    global _NC_CACHE
    assert int(group_size) == GS
    x = np.asarray(x, dtype=np.float32)
    codes = np.asarray(codes, dtype=np.int32)
    codebooks = np.asarray(codebooks, dtype=np.float32)
    scales = np.asarray(scales, dtype=np.float32)

    # ---- host-side layout prep (pure layout/slicing; all math is on-device) ----
    # unpack packed little-endian uint8 code indices: (C, P4, O) int32 -> (C, P, O)
    cb_bytes = codes.view(np.uint8).reshape(C, IN_F // V // 4, OUT_F, 4)
    idx = np.ascontiguousarray(cb_bytes.transpose(0, 1, 3, 2)).reshape(C, P, OUT_F)
    idx_bf16 = idx.astype(ml_dtypes.bfloat16)   # values 0..255: exact in bf16

    # x^T with v-then-pl row order inside each 128-row K-tile:
    # new row kt*128 + pl*8 + v  must hold in_f = kt*128 + pl*8 + v  (natural),
    # but the dequant spread writes W rows in (pl, v) order = natural in_f order,
    # so x^T needs NO permutation beyond the plain transpose.
    xtp = np.ascontiguousarray(x.reshape(T, IN_F).T)

    jcol = np.stack([np.arange(128, dtype=np.float32),
                     np.arange(128, 256, dtype=np.float32)], axis=1)
    cb2 = codebooks.reshape(C * CBN, V)

    if _NC_CACHE is None:
        _NC_CACHE = _build()
    nc = _NC_CACHE

    in_maps = []
    for k in range(NCORE):
        o0, o1 = k * OS, (k + 1) * OS
        in_maps.append({
            "idxsrc": np.ascontiguousarray(idx_bf16[:, :, o0:o1]).reshape(C * P, OS),
            "xtp": xtp,
            "cb": cb2,
            "jcol": jcol,
            "scales": np.ascontiguousarray(scales[:, o0:o1]),
        })

    res = run_bass_kernel_spmd(nc, in_maps, list(range(NCORE)))
    y = np.concatenate([res.results[k]["yT"].T for k in range(NCORE)], axis=1)
    return y.reshape(1, T, OUT_F)
